# revision 12
# baseline (speedup 1.0000x reference)
"""Trainium2 Bass kernel for nn_MixtureOfExperts (B=524288, IN=59, E=4, H=64).

Data-parallel over 8 cores (65536 rows each). Per core, the batch is split
into two halves (A/B) carried on partition ranges 0:60 / 64:124 of a
feature-major x image, processed in 128 windows of 256 columns (512 rows).

Cost-model-driven design (CoreSim v1):
 - matmul cost = out-free-size only, so preds/logits/combine use "flipped"
   matmuls (data tile as stationary lhsT, +/-1 or gw2 patterns moving,
   N=2..4) which are nearly free.
 - stage-1 (5 mm, N=256, f32r) and stage-2 (4 mm bf16 block-diag, or 4
   DoubleRow fp8 mm at half cost) are the only bulk PE work.
 - every PSUM byte must be relu-evicted through Pool/Act/DVE (per-element
   engines); biases are folded into the matmuls (ones-row of x) or into
   the per-partition eviction scalar, and evictions are assigned to the
   three engines by a greedy load balancer at build time.
 - the per-row softmax-combine runs on batched [128,256] "smalls" PSUM
   tiles once per 8 windows: pattern-add (Pool), strided exp (Act),
   mult (Pool), segmented reduce + reciprocal + mul (DVE).
"""

import numpy as np
import ml_dtypes

import concourse.bass as bass
import concourse.mybir as mybir
import concourse.tile as tile
from concourse import bacc
from concourse.bass_utils import run_bass_kernel_spmd

F32 = mybir.dt.float32
F32R = mybir.dt.float32r
BF16 = mybir.dt.bfloat16
FP8 = mybir.dt.float8e4
AF = mybir.ActivationFunctionType
ALU = mybir.AluOpType
DRM = mybir.MatmulPerfMode.DoubleRow

B, IN, E, H, EMB, GH = 524288, 59, 4, 64, 32, 32
EPS = 1e-5
NCORES = 8
BC = B // NCORES            # 65536 rows per core
HB = BC // 2                # 32768 rows per half
WC = 256                    # x columns per window (= 256 A rows + 256 B rows)
NW = HB // WC               # 128 windows
CHW = 8                     # windows per x DMA chunk
NCH = NW // CHW             # 16 chunks
DRW = 8                     # windows per smalls drain
ND = NW // DRW              # 16 drains

VARIANT = "dr"              # "bf16" or "dr" (fp8 DoubleRow stage 2)

_CACHE = {}


def _build(variant):
    nc = bacc.Bacc(trn_type="TRN2")
    x_d = nc.dram_tensor("x", (128, HB), F32R, kind="ExternalInput")
    wts1_d = nc.dram_tensor("wts1", (128, 256), F32R, kind="ExternalInput")
    wg_d = nc.dram_tensor("wg", (128, 64), F32R, kind="ExternalInput")
    w2w = 256 if variant == "bf16" else 512
    w2dt = BF16 if variant == "bf16" else FP8
    h1dt = BF16 if variant == "bf16" else FP8
    w2b_d = nc.dram_tensor("w2b", (128, w2w), w2dt, kind="ExternalInput")
    c2s_d = nc.dram_tensor("c2s", (128, 2), F32, kind="ExternalInput")
    sp_d = nc.dram_tensor("sp", (128, 4), BF16, kind="ExternalInput")
    gw2_d = nc.dram_tensor("gw2t", (32, 8), BF16, kind="ExternalInput")
    pat_d = nc.dram_tensor("pat", (128, 256), F32, kind="ExternalInput")
    out_d = nc.dram_tensor("out", (ND, 128, 32), F32, kind="ExternalOutput")

    # greedy engine balancer for PSUM evictions (costs from the v1 model)
    load = {"pool": 0.0, "act": 0.0, "dve": 0.0}

    def evict(nc, out, in_, bias, cols):
        costs = {
            "pool": 0.833 * cols,
            "act": 0.833 * cols + 185.0,
            "dve": 1.0417 * cols + 125.0,
        }
        eng = min(costs, key=lambda e: load[e] + costs[e])
        load[eng] += costs[eng]
        if eng == "act":
            if bias is None:
                nc.scalar.activation(out, in_, AF.Relu)
            else:
                nc.scalar.activation(out, in_, AF.Relu, bias=bias)
        else:
            e = nc.gpsimd if eng == "pool" else nc.vector
            if bias is None:
                e.tensor_scalar(out, in_, 0.0, None, ALU.max)
            else:
                e.tensor_scalar(out, in_, bias, 0.0, ALU.add, ALU.max)

    with tile.TileContext(nc) as tc:
        with (
            tc.tile_pool(name="consts", bufs=1) as consts,
            tc.tile_pool(name="xs", bufs=3) as xs,
            tc.tile_pool(name="hs", bufs=3) as hs,
            tc.tile_pool(name="ds", bufs=2) as ds,
            tc.tile_pool(name="pha", bufs=2, space="PSUM") as pha,
            tc.tile_pool(name="phb", bufs=2, space="PSUM") as phb,
            tc.tile_pool(name="pg", bufs=2, space="PSUM") as pg,
            tc.tile_pool(name="psm", bufs=2, space="PSUM") as psm,
        ):
            xch = {}
            xt0 = xs.tile([128, CHW * WC], F32R, tag="x")
            nc.sync.dma_start(out=xt0[:, 0:WC], in_=x_d[:, 0:WC])
            xch[0] = (xt0, 0)
            wts1 = consts.tile([128, 256], F32R)
            nc.sync.dma_start(out=wts1, in_=wts1_d[:, :])
            wg = consts.tile([128, 64], F32R)
            nc.sync.dma_start(out=wg, in_=wg_d[:, :])
            w2b = consts.tile([128, w2w], w2dt)
            nc.sync.dma_start(out=w2b, in_=w2b_d[:, :])
            c2s = consts.tile([128, 2], F32)
            nc.sync.dma_start(out=c2s, in_=c2s_d[:, :])
            sp = consts.tile([128, 4], BF16)
            nc.sync.dma_start(out=sp, in_=sp_d[:, :])
            gw2 = consts.tile([32, 8], BF16)
            nc.sync.dma_start(out=gw2, in_=gw2_d[:, :])
            pat = consts.tile([128, 256], F32)
            nc.sync.dma_start(out=pat, in_=pat_d[:, :])

            state = {}
            smt = {}
            for w in range(NW + 4):
                # ---- x chunk prefetch
                if w == 0:
                    for lo, hi in ((1, CHW), (CHW, 2 * CHW)):
                        xt = xs.tile([128, CHW * WC], F32R, tag="x")
                        nc.sync.dma_start(
                            out=xt[:, 0:(hi - lo) * WC],
                            in_=x_d[:, lo * WC:hi * WC])
                        for wi in range(lo, hi):
                            xch[wi] = (xt, wi - lo)
                elif w < NW and w % CHW == 0 and w // CHW + 1 < NCH:
                    ci = w // CHW + 1
                    xt = xs.tile([128, CHW * WC], F32R, tag="x")
                    nc.sync.dma_start(
                        out=xt, in_=x_d[:, ci * CHW * WC:(ci + 1) * CHW * WC])
                    for wi in range(ci * CHW, (ci + 1) * CHW):
                        xch[wi] = (xt, wi - ci * CHW)

                # ---- stage 1 + gating for window w
                if w < NW:
                    xt, lw = xch[w]
                    lc = lw * WC
                    xA = xt[0:60, lc:lc + WC]
                    xB = xt[64:124, lc:lc + WC]
                    pA = pha.tile([128, 512], F32, tag="hA")
                    nc.tensor.matmul(out=pA[:, 0:256], lhsT=wts1[0:60, 0:128],
                                     rhs=xA, start=True, stop=True,
                                     skip_group_check=True)
                    nc.tensor.matmul(out=pA[:, 256:512], lhsT=wts1[0:60, 128:256],
                                     rhs=xA, start=True, stop=True,
                                     skip_group_check=True)
                    pB = phb.tile([128, 512], F32, tag="hB")
                    nc.tensor.matmul(out=pB[:, 0:256], lhsT=wts1[64:124, 0:128],
                                     rhs=xB, start=True, stop=True,
                                     skip_group_check=True)
                    nc.tensor.matmul(out=pB[:, 256:512], lhsT=wts1[64:124, 128:256],
                                     rhs=xB, start=True, stop=True,
                                     skip_group_check=True)
                    if w % 2 == 0:
                        pG = pg.tile([128, 256], F32, tag="g")
                        gpair = [pG, None]
                    else:
                        pG = gpair[0]
                    gbase = 64 * (w % 2)
                    nc.tensor.matmul(out=pG[gbase:gbase + 64, :],
                                     lhsT=wg[0:124, 0:64],
                                     rhs=xt[0:124, lc:lc + WC],
                                     start=True, stop=True,
                                     skip_group_check=True,
                                     tile_position=(0, gbase))
                    h1A = hs.tile([128, 512], h1dt, tag="h1A")
                    evict(nc, h1A, pA, None, 512)
                    h1B = hs.tile([128, 512], h1dt, tag="h1B")
                    evict(nc, h1B, pB, None, 512)
                    if w % 2 == 1:
                        gsb = hs.tile([128, 256], BF16, tag="G")
                        evict(nc, gsb, pG, None, 256)
                        gpair[1] = gsb
                        state[w - 1] = state[w - 1][:2] + (gpair,) + state[w - 1][3:]
                        state[w] = (h1A, h1B, gpair, None, None)
                    else:
                        state[w] = (h1A, h1B, gpair, None, None)

                # ---- stage 2 for window w-2
                if 0 <= w - 2 < NW:
                    h1A, h1B, gsb, _, _ = state[w - 2]
                    p2a = pha.tile([128, 512], F32, tag="hA")
                    p2b = phb.tile([128, 512], F32, tag="hB")
                    if variant == "bf16":
                        for p2, wcol in ((p2a, slice(0, 128)), (p2b, slice(128, 256))):
                            hcol = slice(0, 256) if p2 is p2a else slice(256, 512)
                            nc.tensor.matmul(out=p2[:, 0:256], lhsT=w2b[:, wcol],
                                             rhs=h1A[:, hcol], start=True,
                                             stop=True, skip_group_check=True)
                            nc.tensor.matmul(out=p2[:, 256:512], lhsT=w2b[:, wcol],
                                             rhs=h1B[:, hcol], start=True,
                                             stop=True, skip_group_check=True)
                    else:
                        rA = h1A.rearrange("p (t n) -> p t n", t=2)
                        rB = h1B.rearrange("p (t n) -> p t n", t=2)
                        l02 = w2b[:, 0:256].rearrange("p (t m) -> p t m", t=2)
                        l13 = w2b[:, 256:512].rearrange("p (t m) -> p t m", t=2)
                        for p2, lw in ((p2a, l02), (p2b, l13)):
                            nc.tensor.matmul(out=p2[:, 0:256], lhsT=lw, rhs=rA,
                                             start=True, stop=True, perf_mode=DRM,
                                             skip_group_check=True)
                            nc.tensor.matmul(out=p2[:, 256:512], lhsT=lw, rhs=rB,
                                             start=True, stop=True, perf_mode=DRM,
                                             skip_group_check=True)
                    h2a = hs.tile([128, 512], BF16, tag="h2a")
                    evict(nc, h2a, p2a, c2s[:, 0:1], 512)
                    h2b = hs.tile([128, 512], BF16, tag="h2b")
                    evict(nc, h2b, p2b, c2s[:, 1:2], 512)
                    state[w - 2] = (h1A, h1B, gsb, h2a, h2b)

                # ---- preds/logits (flipped matmuls) for window w-4
                if 0 <= w - 4 < NW:
                    w2i = w - 4
                    _, _, gsb, h2a, h2b = state[w2i]
                    d = w2i // DRW
                    if w2i % DRW == 0:
                        sm_t = psm.tile([128, 256], F32, tag="sm")
                        smt[d] = sm_t
                    sm = smt[d]
                    gsb_t = gsb[1]
                    for sl in range(4):
                        g0 = ((w2i % DRW) * 4 + sl) * 8
                        cl = 128 * sl
                        nc.tensor.matmul(out=sm[:, g0:g0 + 2],
                                         lhsT=h2a[:, cl:cl + 128],
                                         rhs=sp[:, 0:2], start=True, stop=True,
                                         skip_group_check=True,
                                         tile_position=(0, 0))
                        nc.tensor.matmul(out=sm[:, g0 + 2:g0 + 4],
                                         lhsT=h2b[:, cl:cl + 128],
                                         rhs=sp[:, 2:4], start=True, stop=True,
                                         skip_group_check=True,
                                         tile_position=(0, 0))
                        gp = 64 * (w2i % 2) + (0 if sl < 2 else 32)
                        gc = 128 * (sl % 2)
                        nc.tensor.matmul(out=sm[:, g0 + 4:g0 + 8],
                                         lhsT=gsb_t[gp:gp + 32, gc:gc + 128],
                                         rhs=gw2[0:32, 4 * (sl // 2):4 * (sl // 2) + 4],
                                         start=True, stop=True,
                                         skip_group_check=True,
                                         tile_position=(0, 0))
                    del state[w2i]

                    # ---- combine drain once per DRW windows
                    if w2i % DRW == DRW - 1:
                        S = ds.tile([128, 256], F32, tag="S")
                        nc.gpsimd.tensor_tensor(S, sm, pat, op=ALU.add)
                        S4 = S.rearrange("p (g two f) -> p g two f", two=2, f=4)
                        EX = ds.tile([128, 128], F32, tag="EX")
                        nc.scalar.activation(EX, S4[:, :, 1, :], AF.Exp)
                        PW = ds.tile([128, 128], F32, tag="PW")
                        nc.gpsimd.tensor_tensor(PW, S4[:, :, 0, :], EX, op=ALU.mult)
                        NUM = ds.tile([128, 32], F32, tag="NUM")
                        nc.vector.tensor_reduce(
                            NUM, PW.rearrange("p (g f) -> p g f", f=4),
                            mybir.AxisListType.X, ALU.add)
                        DEN = ds.tile([128, 32], F32, tag="DEN")
                        nc.vector.tensor_reduce(
                            DEN, EX.rearrange("p (g f) -> p g f", f=4),
                            mybir.AxisListType.X, ALU.add)
                        REC = ds.tile([128, 32], F32, tag="REC")
                        nc.vector.reciprocal(REC, DEN)
                        OUT = ds.tile([128, 32], F32, tag="OUT")
                        nc.vector.tensor_tensor(OUT, NUM, REC, op=ALU.mult)
                        nc.sync.dma_start(out=out_d[d], in_=OUT)
                        del smt[d]

    if not nc.is_finalized():
        nc.finalize()
    return nc


def _pack_host(variant, w1, b1, bn1_g, bn1_b, bn1_m, bn1_v, w2, b2, bn2_g,
               bn2_b, bn2_m, bn2_v, w3, b3, wp, bp, gw1, gb1, gw2, gb2):
    f = np.float32
    s1 = (bn1_g / np.sqrt(bn1_v + EPS)).astype(f)               # (E,H)
    W1e = (w1 * s1[:, None, :]).astype(f)                       # (E,IN,H)
    c1 = ((b1 - bn1_m) * s1 + bn1_b).astype(f)                  # (E,H)
    s2f = (bn2_g / np.sqrt(bn2_v + EPS)).astype(f)
    W2e = (w2 * s2f[:, None, :]).astype(f)                      # (E,H,H)
    c2 = ((b2 - bn2_m) * s2f + bn2_b).astype(f)                 # (E,H)
    wep = np.einsum("ehm,em->eh", w3, wp).astype(f)             # (E,H)
    bep = (np.einsum("em,em->e", b3, wp) + bp).astype(f)        # (E,)
    aw = np.abs(wep)
    sg = np.sign(wep).astype(f)
    W2p = (W2e * aw[:, None, :]).astype(f)                      # (E,H,H)
    c2p = (c2 * aw).astype(f)                                   # (E,H)

    wts1 = np.zeros((128, 256), f)
    for blk, (ea, eb) in enumerate(((0, 1), (2, 3))):
        c0 = 128 * blk
        for base in (0, 64):
            wts1[base:base + 59, c0:c0 + 64] = W1e[ea]
            wts1[base:base + 59, c0 + 64:c0 + 128] = W1e[eb]
            wts1[base + 59, c0:c0 + 64] = c1[ea]
            wts1[base + 59, c0 + 64:c0 + 128] = c1[eb]

    wgp = np.zeros((128, 64), f)
    wgp[0:59, 0:32] = gw1
    wgp[59, 0:32] = gb1
    wgp[64:123, 32:64] = gw1
    wgp[123, 32:64] = gb1

    if variant == "bf16":
        po = [0, 1, 2, 3]
        w2b = np.zeros((128, 256), f)
        w2b[0:64, 0:64] = W2p[0]
        w2b[64:128, 64:128] = W2p[1]
        w2b[0:64, 128:192] = W2p[2]
        w2b[64:128, 192:256] = W2p[3]
        c2sv = np.stack([np.concatenate([c2p[0], c2p[1]]),
                         np.concatenate([c2p[2], c2p[3]])], axis=1)
        spv = np.zeros((128, 4), f)
        spv[0:64, 0] = sg[0]
        spv[64:128, 1] = sg[1]
        spv[0:64, 2] = sg[2]
        spv[64:128, 3] = sg[3]
    else:
        po = [0, 2, 1, 3]
        # keep fp8 weights out of the e4m3 subnormal zone; the 2**-10 is
        # folded into the +/-1 pred patterns (exact in bf16)
        FSC = 1024.0
        W2p = W2p * FSC
        c2p = c2p * FSC
        sg = sg / FSC
        t02 = np.zeros((128, 2, 128), f)
        t02[0:64, 0, 0:64] = W2p[0]
        t02[0:64, 1, 64:128] = W2p[2]
        t13 = np.zeros((128, 2, 128), f)
        t13[64:128, 0, 0:64] = W2p[1]
        t13[64:128, 1, 64:128] = W2p[3]
        w2b = np.concatenate([t02.reshape(128, 256), t13.reshape(128, 256)],
                             axis=1)
        c2sv = np.stack([np.concatenate([c2p[0], c2p[2]]),
                         np.concatenate([c2p[1], c2p[3]])], axis=1)
        spv = np.zeros((128, 4), f)
        spv[0:64, 0] = sg[0]
        spv[64:128, 1] = sg[2]
        spv[0:64, 2] = sg[1]
        spv[64:128, 3] = sg[3]

    gw2t = np.zeros((32, 8), f)
    gw2t[:, 0:4] = gw2[:, po]
    gw2t[:, 4:8] = gw2[:, po]

    grp = np.concatenate([bep[po], gb2[po]]).astype(f)          # (8,)
    patv = np.tile(grp, 32)[None, :].repeat(128, axis=0)

    w2dt = ml_dtypes.bfloat16 if variant == "bf16" else ml_dtypes.float8_e4m3fn
    return dict(
        wts1=np.ascontiguousarray(wts1),
        wg=np.ascontiguousarray(wgp),
        w2b=np.ascontiguousarray(w2b.astype(w2dt)),
        c2s=np.ascontiguousarray(c2sv.astype(f)),
        sp=np.ascontiguousarray(spv.astype(ml_dtypes.bfloat16)),
        gw2t=np.ascontiguousarray(gw2t.astype(ml_dtypes.bfloat16)),
        pat=np.ascontiguousarray(patv),
    )


def _pack_x_core(xc):
    # xc: (BC, 59) float32 -> feature-major image (128, HB)
    xi = np.zeros((128, HB), np.float32)
    xi[0:59] = xc[:HB].T
    xi[59] = 1.0
    xi[64:123] = xc[HB:].T
    xi[123] = 1.0
    return np.ascontiguousarray(xi)


def _unshard_core(o):
    # o: (ND, 128, 32) -> (BC,) predictions
    v = o.reshape(ND, 128, DRW, 4).transpose(0, 2, 3, 1)  # (d, wl, sl, p)
    a = v[:, :, 0:2, :].reshape(HB)
    b = v[:, :, 2:4, :].reshape(HB)
    return np.concatenate([a, b])


def kernel(**inputs):
    x = np.asarray(inputs["x"], dtype=np.float32)
    wk = {k: np.asarray(v, dtype=np.float32) for k, v in inputs.items()
          if k != "x"}
    packed = _pack_host(VARIANT, **wk)

    key = "nc_" + VARIANT
    if key not in _CACHE:
        _CACHE[key] = _build(VARIANT)
    nc = _CACHE[key]

    in_maps = []
    for c in range(NCORES):
        m = {"x": _pack_x_core(x[c * BC:(c + 1) * BC])}
        m.update(packed)
        in_maps.append(m)
    _CACHE["in_maps"] = in_maps

    res = run_bass_kernel_spmd(nc, in_maps, core_ids=list(range(NCORES)))
    _CACHE["last"] = res
    outs = [_unshard_core(r["out"]) for r in res.results]
    return np.concatenate(outs).reshape(B, 1).astype(np.float32)


# revision 13
# speedup vs baseline: 1.0143x; 1.0143x over previous
"""Trainium2 Bass kernel for nn_MixtureOfExperts (B=524288, IN=59, E=4, H=64).

Data-parallel over 8 cores (65536 rows each). Per core, the batch is split
into two halves (A/B) carried on partition ranges 0:60 / 64:124 of a
feature-major x image, processed in 128 windows of 256 columns (512 rows).

Cost-model-driven design (CoreSim v1):
 - matmul cost = out-free-size only, so preds/logits/combine use "flipped"
   matmuls (data tile as stationary lhsT, +/-1 or gw2 patterns moving,
   N=2..4) which are nearly free.
 - stage-1 (5 mm, N=256, f32r) and stage-2 (4 mm bf16 block-diag, or 4
   DoubleRow fp8 mm at half cost) are the only bulk PE work.
 - every PSUM byte must be relu-evicted through Pool/Act/DVE (per-element
   engines); biases are folded into the matmuls (ones-row of x) or into
   the per-partition eviction scalar, and evictions are assigned to the
   three engines by a greedy load balancer at build time.
 - the per-row softmax-combine runs on batched [128,256] "smalls" PSUM
   tiles once per 8 windows: pattern-add (Pool), strided exp (Act),
   mult (Pool), segmented reduce + reciprocal + mul (DVE).
"""

import numpy as np
import ml_dtypes

import concourse.bass as bass
import concourse.mybir as mybir
import concourse.tile as tile
from concourse import bacc
from concourse.bass_utils import run_bass_kernel_spmd

F32 = mybir.dt.float32
F32R = mybir.dt.float32r
BF16 = mybir.dt.bfloat16
FP8 = mybir.dt.float8e4
AF = mybir.ActivationFunctionType
ALU = mybir.AluOpType
DRM = mybir.MatmulPerfMode.DoubleRow

B, IN, E, H, EMB, GH = 524288, 59, 4, 64, 32, 32
EPS = 1e-5
NCORES = 8
BC = B // NCORES            # 65536 rows per core
HB = BC // 2                # 32768 rows per half
WC = 256                    # x columns per window (= 256 A rows + 256 B rows)
NW = HB // WC               # 128 windows
CHW = 8                     # windows per x DMA chunk
NCH = NW // CHW             # 16 chunks
DRW = 8                     # windows per smalls drain
ND = NW // DRW              # 16 drains

VARIANT = "dr"              # "bf16" or "dr" (fp8 DoubleRow stage 2)

_CACHE = {}


def _build(variant):
    nc = bacc.Bacc(trn_type="TRN2")
    x_d = nc.dram_tensor("x", (128, HB), F32R, kind="ExternalInput")
    wts1_d = nc.dram_tensor("wts1", (128, 256), F32R, kind="ExternalInput")
    wg_d = nc.dram_tensor("wg", (128, 64), F32R, kind="ExternalInput")
    w2w = 256 if variant == "bf16" else 512
    w2dt = BF16 if variant == "bf16" else FP8
    h1dt = BF16 if variant == "bf16" else FP8
    w2b_d = nc.dram_tensor("w2b", (128, w2w), w2dt, kind="ExternalInput")
    c2s_d = nc.dram_tensor("c2s", (128, 2), F32, kind="ExternalInput")
    sp_d = nc.dram_tensor("sp", (128, 4), BF16, kind="ExternalInput")
    gw2_d = nc.dram_tensor("gw2t", (32, 8), BF16, kind="ExternalInput")
    pat_d = nc.dram_tensor("pat", (128, 256), F32, kind="ExternalInput")
    out_d = nc.dram_tensor("out", (ND, 128, 32), F32, kind="ExternalOutput")

    # greedy engine balancer for PSUM evictions (costs from the v1 model)
    load = {"pool": 0.0, "act": 0.0, "dve": 0.0}

    def evict(nc, out, in_, bias, cols):
        costs = {
            "pool": 0.833 * cols,
            "act": 0.833 * cols + 185.0,
            "dve": 1.0417 * cols + 125.0,
        }
        eng = min(costs, key=lambda e: load[e] + costs[e])
        load[eng] += costs[eng]
        if eng == "act":
            if bias is None:
                nc.scalar.activation(out, in_, AF.Relu)
            else:
                nc.scalar.activation(out, in_, AF.Relu, bias=bias)
        else:
            e = nc.gpsimd if eng == "pool" else nc.vector
            if bias is None:
                e.tensor_scalar(out, in_, 0.0, None, ALU.max)
            else:
                e.tensor_scalar(out, in_, bias, 0.0, ALU.add, ALU.max)

    with tile.TileContext(nc) as tc:
        with (
            tc.tile_pool(name="consts", bufs=1) as consts,
            tc.tile_pool(name="xs", bufs=3) as xs,
            tc.tile_pool(name="hs", bufs=3) as hs,
            tc.tile_pool(name="ds", bufs=2) as ds,
            tc.tile_pool(name="pha", bufs=2, space="PSUM") as pha,
            tc.tile_pool(name="phb", bufs=2, space="PSUM") as phb,
            tc.tile_pool(name="pg", bufs=2, space="PSUM") as pg,
            tc.tile_pool(name="psm", bufs=2, space="PSUM") as psm,
        ):
            wts1 = consts.tile([128, 256], F32R)
            nc.sync.dma_start(out=wts1, in_=wts1_d[:, :])
            wg = consts.tile([128, 64], F32R)
            nc.sync.dma_start(out=wg, in_=wg_d[:, :])
            w2b = consts.tile([128, w2w], w2dt)
            nc.sync.dma_start(out=w2b, in_=w2b_d[:, :])
            c2s = consts.tile([128, 2], F32)
            nc.sync.dma_start(out=c2s, in_=c2s_d[:, :])
            sp = consts.tile([128, 4], BF16)
            nc.sync.dma_start(out=sp, in_=sp_d[:, :])
            gw2 = consts.tile([32, 8], BF16)
            nc.sync.dma_start(out=gw2, in_=gw2_d[:, :])
            pat = consts.tile([128, 256], F32)
            nc.sync.dma_start(out=pat, in_=pat_d[:, :])

            state = {}
            xch = {}
            smt = {}
            for w in range(NW + 4):
                # ---- x chunk prefetch
                if w == 0:
                    for lo, hi in ((0, 1), (1, CHW), (CHW, 2 * CHW)):
                        xt = xs.tile([128, CHW * WC], F32R, tag="x")
                        nc.sync.dma_start(
                            out=xt[:, 0:(hi - lo) * WC],
                            in_=x_d[:, lo * WC:hi * WC])
                        for wi in range(lo, hi):
                            xch[wi] = (xt, wi - lo)
                elif w < NW and w % CHW == 0 and w // CHW + 1 < NCH:
                    ci = w // CHW + 1
                    xt = xs.tile([128, CHW * WC], F32R, tag="x")
                    nc.sync.dma_start(
                        out=xt, in_=x_d[:, ci * CHW * WC:(ci + 1) * CHW * WC])
                    for wi in range(ci * CHW, (ci + 1) * CHW):
                        xch[wi] = (xt, wi - ci * CHW)

                # ---- stage 1 + gating for window w
                if w < NW:
                    xt, lw = xch[w]
                    lc = lw * WC
                    xA = xt[0:60, lc:lc + WC]
                    xB = xt[64:124, lc:lc + WC]
                    pA = pha.tile([128, 512], F32, tag="hA")
                    nc.tensor.matmul(out=pA[:, 0:256], lhsT=wts1[0:60, 0:128],
                                     rhs=xA, start=True, stop=True,
                                     skip_group_check=True)
                    nc.tensor.matmul(out=pA[:, 256:512], lhsT=wts1[0:60, 128:256],
                                     rhs=xA, start=True, stop=True,
                                     skip_group_check=True)
                    pB = phb.tile([128, 512], F32, tag="hB")
                    nc.tensor.matmul(out=pB[:, 0:256], lhsT=wts1[64:124, 0:128],
                                     rhs=xB, start=True, stop=True,
                                     skip_group_check=True)
                    nc.tensor.matmul(out=pB[:, 256:512], lhsT=wts1[64:124, 128:256],
                                     rhs=xB, start=True, stop=True,
                                     skip_group_check=True)
                    if w % 2 == 0:
                        pG = pg.tile([128, 256], F32, tag="g")
                        gpair = [pG, None]
                    else:
                        pG = gpair[0]
                    gbase = 64 * (w % 2)
                    nc.tensor.matmul(out=pG[gbase:gbase + 64, :],
                                     lhsT=wg[0:124, 0:64],
                                     rhs=xt[0:124, lc:lc + WC],
                                     start=True, stop=True,
                                     skip_group_check=True,
                                     tile_position=(0, gbase))
                    h1A = hs.tile([128, 512], h1dt, tag="h1A")
                    evict(nc, h1A, pA, None, 512)
                    h1B = hs.tile([128, 512], h1dt, tag="h1B")
                    evict(nc, h1B, pB, None, 512)
                    if w % 2 == 1:
                        gsb = hs.tile([128, 256], BF16, tag="G")
                        evict(nc, gsb, pG, None, 256)
                        gpair[1] = gsb
                        state[w - 1] = state[w - 1][:2] + (gpair,) + state[w - 1][3:]
                        state[w] = (h1A, h1B, gpair, None, None)
                    else:
                        state[w] = (h1A, h1B, gpair, None, None)

                # ---- stage 2 for window w-2
                if 0 <= w - 2 < NW:
                    h1A, h1B, gsb, _, _ = state[w - 2]
                    p2a = pha.tile([128, 512], F32, tag="hA")
                    p2b = phb.tile([128, 512], F32, tag="hB")
                    if variant == "bf16":
                        for p2, wcol in ((p2a, slice(0, 128)), (p2b, slice(128, 256))):
                            hcol = slice(0, 256) if p2 is p2a else slice(256, 512)
                            nc.tensor.matmul(out=p2[:, 0:256], lhsT=w2b[:, wcol],
                                             rhs=h1A[:, hcol], start=True,
                                             stop=True, skip_group_check=True)
                            nc.tensor.matmul(out=p2[:, 256:512], lhsT=w2b[:, wcol],
                                             rhs=h1B[:, hcol], start=True,
                                             stop=True, skip_group_check=True)
                    else:
                        rA = h1A.rearrange("p (t n) -> p t n", t=2)
                        rB = h1B.rearrange("p (t n) -> p t n", t=2)
                        l02 = w2b[:, 0:256].rearrange("p (t m) -> p t m", t=2)
                        l13 = w2b[:, 256:512].rearrange("p (t m) -> p t m", t=2)
                        for p2, lw in ((p2a, l02), (p2b, l13)):
                            nc.tensor.matmul(out=p2[:, 0:256], lhsT=lw, rhs=rA,
                                             start=True, stop=True, perf_mode=DRM,
                                             skip_group_check=True)
                            nc.tensor.matmul(out=p2[:, 256:512], lhsT=lw, rhs=rB,
                                             start=True, stop=True, perf_mode=DRM,
                                             skip_group_check=True)
                    h2a = hs.tile([128, 512], BF16, tag="h2a")
                    evict(nc, h2a, p2a, c2s[:, 0:1], 512)
                    h2b = hs.tile([128, 512], BF16, tag="h2b")
                    evict(nc, h2b, p2b, c2s[:, 1:2], 512)
                    state[w - 2] = (h1A, h1B, gsb, h2a, h2b)

                # ---- preds/logits (flipped matmuls) for window w-4
                if 0 <= w - 4 < NW:
                    w2i = w - 4
                    _, _, gsb, h2a, h2b = state[w2i]
                    d = w2i // DRW
                    if w2i % DRW == 0:
                        sm_t = psm.tile([128, 256], F32, tag="sm")
                        smt[d] = sm_t
                    sm = smt[d]
                    gsb_t = gsb[1]
                    for sl in range(4):
                        g0 = ((w2i % DRW) * 4 + sl) * 8
                        cl = 128 * sl
                        nc.tensor.matmul(out=sm[:, g0:g0 + 2],
                                         lhsT=h2a[:, cl:cl + 128],
                                         rhs=sp[:, 0:2], start=True, stop=True,
                                         skip_group_check=True,
                                         tile_position=(0, 0))
                        nc.tensor.matmul(out=sm[:, g0 + 2:g0 + 4],
                                         lhsT=h2b[:, cl:cl + 128],
                                         rhs=sp[:, 2:4], start=True, stop=True,
                                         skip_group_check=True,
                                         tile_position=(0, 0))
                        gp = 64 * (w2i % 2) + (0 if sl < 2 else 32)
                        gc = 128 * (sl % 2)
                        nc.tensor.matmul(out=sm[:, g0 + 4:g0 + 8],
                                         lhsT=gsb_t[gp:gp + 32, gc:gc + 128],
                                         rhs=gw2[0:32, 4 * (sl // 2):4 * (sl // 2) + 4],
                                         start=True, stop=True,
                                         skip_group_check=True,
                                         tile_position=(0, 0))
                    del state[w2i]

                    # ---- combine drain once per DRW windows
                    if w2i % DRW == DRW - 1:
                        S = ds.tile([128, 256], F32, tag="S")
                        nc.gpsimd.tensor_tensor(S, sm, pat, op=ALU.add)
                        S4 = S.rearrange("p (g two f) -> p g two f", two=2, f=4)
                        EX = ds.tile([128, 128], F32, tag="EX")
                        nc.scalar.activation(EX, S4[:, :, 1, :], AF.Exp)
                        PW = ds.tile([128, 128], F32, tag="PW")
                        nc.gpsimd.tensor_tensor(PW, S4[:, :, 0, :], EX, op=ALU.mult)
                        NUM = ds.tile([128, 32], F32, tag="NUM")
                        nc.vector.tensor_reduce(
                            NUM, PW.rearrange("p (g f) -> p g f", f=4),
                            mybir.AxisListType.X, ALU.add)
                        DEN = ds.tile([128, 32], F32, tag="DEN")
                        nc.vector.tensor_reduce(
                            DEN, EX.rearrange("p (g f) -> p g f", f=4),
                            mybir.AxisListType.X, ALU.add)
                        REC = ds.tile([128, 32], F32, tag="REC")
                        nc.vector.reciprocal(REC, DEN)
                        OUT = ds.tile([128, 32], F32, tag="OUT")
                        nc.vector.tensor_tensor(OUT, NUM, REC, op=ALU.mult)
                        nc.sync.dma_start(out=out_d[d], in_=OUT)
                        del smt[d]

    if not nc.is_finalized():
        nc.finalize()
    return nc


def _pack_host(variant, w1, b1, bn1_g, bn1_b, bn1_m, bn1_v, w2, b2, bn2_g,
               bn2_b, bn2_m, bn2_v, w3, b3, wp, bp, gw1, gb1, gw2, gb2):
    f = np.float32
    s1 = (bn1_g / np.sqrt(bn1_v + EPS)).astype(f)               # (E,H)
    W1e = (w1 * s1[:, None, :]).astype(f)                       # (E,IN,H)
    c1 = ((b1 - bn1_m) * s1 + bn1_b).astype(f)                  # (E,H)
    s2f = (bn2_g / np.sqrt(bn2_v + EPS)).astype(f)
    W2e = (w2 * s2f[:, None, :]).astype(f)                      # (E,H,H)
    c2 = ((b2 - bn2_m) * s2f + bn2_b).astype(f)                 # (E,H)
    wep = np.einsum("ehm,em->eh", w3, wp).astype(f)             # (E,H)
    bep = (np.einsum("em,em->e", b3, wp) + bp).astype(f)        # (E,)
    aw = np.abs(wep)
    sg = np.sign(wep).astype(f)
    W2p = (W2e * aw[:, None, :]).astype(f)                      # (E,H,H)
    c2p = (c2 * aw).astype(f)                                   # (E,H)

    wts1 = np.zeros((128, 256), f)
    for blk, (ea, eb) in enumerate(((0, 1), (2, 3))):
        c0 = 128 * blk
        for base in (0, 64):
            wts1[base:base + 59, c0:c0 + 64] = W1e[ea]
            wts1[base:base + 59, c0 + 64:c0 + 128] = W1e[eb]
            wts1[base + 59, c0:c0 + 64] = c1[ea]
            wts1[base + 59, c0 + 64:c0 + 128] = c1[eb]

    wgp = np.zeros((128, 64), f)
    wgp[0:59, 0:32] = gw1
    wgp[59, 0:32] = gb1
    wgp[64:123, 32:64] = gw1
    wgp[123, 32:64] = gb1

    if variant == "bf16":
        po = [0, 1, 2, 3]
        w2b = np.zeros((128, 256), f)
        w2b[0:64, 0:64] = W2p[0]
        w2b[64:128, 64:128] = W2p[1]
        w2b[0:64, 128:192] = W2p[2]
        w2b[64:128, 192:256] = W2p[3]
        c2sv = np.stack([np.concatenate([c2p[0], c2p[1]]),
                         np.concatenate([c2p[2], c2p[3]])], axis=1)
        spv = np.zeros((128, 4), f)
        spv[0:64, 0] = sg[0]
        spv[64:128, 1] = sg[1]
        spv[0:64, 2] = sg[2]
        spv[64:128, 3] = sg[3]
    else:
        po = [0, 2, 1, 3]
        # keep fp8 weights out of the e4m3 subnormal zone; the 2**-10 is
        # folded into the +/-1 pred patterns (exact in bf16)
        FSC = 1024.0
        W2p = W2p * FSC
        c2p = c2p * FSC
        sg = sg / FSC
        t02 = np.zeros((128, 2, 128), f)
        t02[0:64, 0, 0:64] = W2p[0]
        t02[0:64, 1, 64:128] = W2p[2]
        t13 = np.zeros((128, 2, 128), f)
        t13[64:128, 0, 0:64] = W2p[1]
        t13[64:128, 1, 64:128] = W2p[3]
        w2b = np.concatenate([t02.reshape(128, 256), t13.reshape(128, 256)],
                             axis=1)
        c2sv = np.stack([np.concatenate([c2p[0], c2p[2]]),
                         np.concatenate([c2p[1], c2p[3]])], axis=1)
        spv = np.zeros((128, 4), f)
        spv[0:64, 0] = sg[0]
        spv[64:128, 1] = sg[2]
        spv[0:64, 2] = sg[1]
        spv[64:128, 3] = sg[3]

    gw2t = np.zeros((32, 8), f)
    gw2t[:, 0:4] = gw2[:, po]
    gw2t[:, 4:8] = gw2[:, po]

    grp = np.concatenate([bep[po], gb2[po]]).astype(f)          # (8,)
    patv = np.tile(grp, 32)[None, :].repeat(128, axis=0)

    w2dt = ml_dtypes.bfloat16 if variant == "bf16" else ml_dtypes.float8_e4m3fn
    return dict(
        wts1=np.ascontiguousarray(wts1),
        wg=np.ascontiguousarray(wgp),
        w2b=np.ascontiguousarray(w2b.astype(w2dt)),
        c2s=np.ascontiguousarray(c2sv.astype(f)),
        sp=np.ascontiguousarray(spv.astype(ml_dtypes.bfloat16)),
        gw2t=np.ascontiguousarray(gw2t.astype(ml_dtypes.bfloat16)),
        pat=np.ascontiguousarray(patv),
    )


def _pack_x_core(xc):
    # xc: (BC, 59) float32 -> feature-major image (128, HB)
    xi = np.zeros((128, HB), np.float32)
    xi[0:59] = xc[:HB].T
    xi[59] = 1.0
    xi[64:123] = xc[HB:].T
    xi[123] = 1.0
    return np.ascontiguousarray(xi)


def _unshard_core(o):
    # o: (ND, 128, 32) -> (BC,) predictions
    v = o.reshape(ND, 128, DRW, 4).transpose(0, 2, 3, 1)  # (d, wl, sl, p)
    a = v[:, :, 0:2, :].reshape(HB)
    b = v[:, :, 2:4, :].reshape(HB)
    return np.concatenate([a, b])


def kernel(**inputs):
    x = np.asarray(inputs["x"], dtype=np.float32)
    wk = {k: np.asarray(v, dtype=np.float32) for k, v in inputs.items()
          if k != "x"}
    packed = _pack_host(VARIANT, **wk)

    key = "nc_" + VARIANT
    if key not in _CACHE:
        _CACHE[key] = _build(VARIANT)
    nc = _CACHE[key]

    in_maps = []
    for c in range(NCORES):
        m = {"x": _pack_x_core(x[c * BC:(c + 1) * BC])}
        m.update(packed)
        in_maps.append(m)
    _CACHE["in_maps"] = in_maps

    res = run_bass_kernel_spmd(nc, in_maps, core_ids=list(range(NCORES)))
    _CACHE["last"] = res
    outs = [_unshard_core(r["out"]) for r in res.results]
    return np.concatenate(outs).reshape(B, 1).astype(np.float32)


# revision 15
# speedup vs baseline: 1.0270x; 1.0125x over previous
"""Trainium2 Bass kernel for nn_MixtureOfExperts (B=524288, IN=59, E=4, H=64).

Data-parallel over 8 cores (65536 rows each). Per core, the batch is split
into two halves (A/B) carried on partition ranges 0:60 / 64:124 of a
feature-major x image, processed in 128 windows of 256 columns (512 rows).

Cost-model-driven design (CoreSim v1):
 - matmul cost = out-free-size only, so preds/logits/combine use "flipped"
   matmuls (data tile as stationary lhsT, +/-1 or gw2 patterns moving,
   N=2..4) which are nearly free.
 - stage-1 (5 mm, N=256, f32r) and stage-2 (4 mm bf16 block-diag, or 4
   DoubleRow fp8 mm at half cost) are the only bulk PE work.
 - every PSUM byte must be relu-evicted through Pool/Act/DVE (per-element
   engines); biases are folded into the matmuls (ones-row of x) or into
   the per-partition eviction scalar, and evictions are assigned to the
   three engines by a greedy load balancer at build time.
 - the per-row softmax-combine runs on batched [128,256] "smalls" PSUM
   tiles once per 8 windows: pattern-add (Pool), strided exp (Act),
   mult (Pool), segmented reduce + reciprocal + mul (DVE).
"""

import numpy as np
import ml_dtypes

import concourse.bass as bass
import concourse.mybir as mybir
import concourse.tile as tile
from concourse import bacc
from concourse.bass_utils import run_bass_kernel_spmd

F32 = mybir.dt.float32
F32R = mybir.dt.float32r
BF16 = mybir.dt.bfloat16
FP8 = mybir.dt.float8e4
AF = mybir.ActivationFunctionType
ALU = mybir.AluOpType
DRM = mybir.MatmulPerfMode.DoubleRow

B, IN, E, H, EMB, GH = 524288, 59, 4, 64, 32, 32
EPS = 1e-5
NCORES = 8
BC = B // NCORES            # 65536 rows per core
HB = BC // 2                # 32768 rows per half
WC = 256                    # x columns per window (= 256 A rows + 256 B rows)
NW = HB // WC               # 128 windows
CHW = 8                     # windows per x DMA chunk
NCH = NW // CHW             # 16 chunks
DRW = 8                     # windows per smalls drain
ND = NW // DRW              # 16 drains

VARIANT = "bf16"          # "bf16" or "dr" (fp8 DoubleRow stage 2)

_CACHE = {}


def _build(variant):
    nc = bacc.Bacc(trn_type="TRN2")
    x_d = nc.dram_tensor("x", (128, HB), F32R, kind="ExternalInput")
    wts1_d = nc.dram_tensor("wts1", (128, 256), F32R, kind="ExternalInput")
    wg_d = nc.dram_tensor("wg", (128, 64), F32R, kind="ExternalInput")
    w2w = 256 if variant == "bf16" else 512
    w2dt = BF16 if variant == "bf16" else FP8
    h1dt = BF16 if variant == "bf16" else FP8
    w2b_d = nc.dram_tensor("w2b", (128, w2w), w2dt, kind="ExternalInput")
    c2s_d = nc.dram_tensor("c2s", (128, 2), F32, kind="ExternalInput")
    sp_d = nc.dram_tensor("sp", (128, 4), BF16, kind="ExternalInput")
    gw2_d = nc.dram_tensor("gw2t", (32, 8), BF16, kind="ExternalInput")
    pat_d = nc.dram_tensor("pat", (128, 256), F32, kind="ExternalInput")
    out_d = nc.dram_tensor("out", (ND, 128, 32), F32, kind="ExternalOutput")

    # greedy engine balancer for PSUM evictions (costs from the v1 model)
    load = {"pool": 0.0, "act": 0.0, "dve": 0.0}

    def evict(nc, out, in_, bias, cols):
        costs = {
            "pool": 0.833 * cols,
            "act": 0.833 * cols + 185.0,
            "dve": 1.0417 * cols + 125.0,
        }
        eng = min(costs, key=lambda e: load[e] + costs[e])
        load[eng] += costs[eng]
        if eng == "act":
            if bias is None:
                nc.scalar.activation(out, in_, AF.Relu)
            else:
                nc.scalar.activation(out, in_, AF.Relu, bias=bias)
        else:
            e = nc.gpsimd if eng == "pool" else nc.vector
            if bias is None:
                e.tensor_scalar(out, in_, 0.0, None, ALU.max)
            else:
                e.tensor_scalar(out, in_, bias, 0.0, ALU.add, ALU.max)

    with tile.TileContext(nc) as tc:
        with (
            tc.tile_pool(name="consts", bufs=1) as consts,
            tc.tile_pool(name="xs", bufs=3) as xs,
            tc.tile_pool(name="hs", bufs=3) as hs,
            tc.tile_pool(name="ds", bufs=2) as ds,
            tc.tile_pool(name="pha", bufs=2, space="PSUM") as pha,
            tc.tile_pool(name="phb", bufs=2, space="PSUM") as phb,
            tc.tile_pool(name="pg", bufs=2, space="PSUM") as pg,
            tc.tile_pool(name="psm", bufs=2, space="PSUM") as psm,
        ):
            wts1 = consts.tile([128, 256], F32R)
            nc.sync.dma_start(out=wts1, in_=wts1_d[:, :])
            wg = consts.tile([128, 64], F32R)
            nc.sync.dma_start(out=wg, in_=wg_d[:, :])
            w2b = consts.tile([128, w2w], w2dt)
            nc.sync.dma_start(out=w2b, in_=w2b_d[:, :])
            c2s = consts.tile([128, 2], F32)
            nc.sync.dma_start(out=c2s, in_=c2s_d[:, :])
            sp = consts.tile([128, 4], BF16)
            nc.sync.dma_start(out=sp, in_=sp_d[:, :])
            gw2 = consts.tile([32, 8], BF16)
            nc.sync.dma_start(out=gw2, in_=gw2_d[:, :])
            pat = consts.tile([128, 256], F32)
            nc.sync.dma_start(out=pat, in_=pat_d[:, :])

            state = {}
            xch = {}
            smt = {}
            for w in range(NW + 4):
                # ---- x chunk prefetch
                if w == 0:
                    for lo, hi in ((0, 1), (1, CHW), (CHW, 2 * CHW)):
                        xt = xs.tile([128, CHW * WC], F32R, tag="x")
                        nc.sync.dma_start(
                            out=xt[:, 0:(hi - lo) * WC],
                            in_=x_d[:, lo * WC:hi * WC])
                        for wi in range(lo, hi):
                            xch[wi] = (xt, wi - lo)
                elif w < NW and w % CHW == 0 and w // CHW + 1 < NCH:
                    ci = w // CHW + 1
                    xt = xs.tile([128, CHW * WC], F32R, tag="x")
                    nc.sync.dma_start(
                        out=xt, in_=x_d[:, ci * CHW * WC:(ci + 1) * CHW * WC])
                    for wi in range(ci * CHW, (ci + 1) * CHW):
                        xch[wi] = (xt, wi - ci * CHW)

                # ---- stage 2 for window w-2
                if 0 <= w - 2 < NW:
                    h1A, h1B, gsb, _, _ = state[w - 2]
                    p2a = pha.tile([128, 512], F32, tag="hA")
                    p2b = phb.tile([128, 512], F32, tag="hB")
                    if variant == "bf16":
                        for p2, wcol in ((p2a, slice(0, 128)), (p2b, slice(128, 256))):
                            hcol = slice(0, 256) if p2 is p2a else slice(256, 512)
                            nc.tensor.matmul(out=p2[:, 0:256], lhsT=w2b[:, wcol],
                                             rhs=h1A[:, hcol], start=True,
                                             stop=True, skip_group_check=True)
                            nc.tensor.matmul(out=p2[:, 256:512], lhsT=w2b[:, wcol],
                                             rhs=h1B[:, hcol], start=True,
                                             stop=True, skip_group_check=True)
                    else:
                        rA = h1A.rearrange("p (t n) -> p t n", t=2)
                        rB = h1B.rearrange("p (t n) -> p t n", t=2)
                        l02 = w2b[:, 0:256].rearrange("p (t m) -> p t m", t=2)
                        l13 = w2b[:, 256:512].rearrange("p (t m) -> p t m", t=2)
                        for p2, lw in ((p2a, l02), (p2b, l13)):
                            nc.tensor.matmul(out=p2[:, 0:256], lhsT=lw, rhs=rA,
                                             start=True, stop=True, perf_mode=DRM,
                                             skip_group_check=True)
                            nc.tensor.matmul(out=p2[:, 256:512], lhsT=lw, rhs=rB,
                                             start=True, stop=True, perf_mode=DRM,
                                             skip_group_check=True)
                    h2a = hs.tile([128, 512], BF16, tag="h2a")
                    evict(nc, h2a, p2a, c2s[:, 0:1], 512)
                    h2b = hs.tile([128, 512], BF16, tag="h2b")
                    evict(nc, h2b, p2b, c2s[:, 1:2], 512)
                    state[w - 2] = (h1A, h1B, gsb, h2a, h2b)

                # ---- stage 1 + gating for window w
                if w < NW:
                    xt, lw = xch[w]
                    lc = lw * WC
                    xA = xt[0:60, lc:lc + WC]
                    xB = xt[64:124, lc:lc + WC]
                    pA = pha.tile([128, 512], F32, tag="hA")
                    nc.tensor.matmul(out=pA[:, 0:256], lhsT=wts1[0:60, 0:128],
                                     rhs=xA, start=True, stop=True,
                                     skip_group_check=True)
                    nc.tensor.matmul(out=pA[:, 256:512], lhsT=wts1[0:60, 128:256],
                                     rhs=xA, start=True, stop=True,
                                     skip_group_check=True)
                    pB = phb.tile([128, 512], F32, tag="hB")
                    nc.tensor.matmul(out=pB[:, 0:256], lhsT=wts1[64:124, 0:128],
                                     rhs=xB, start=True, stop=True,
                                     skip_group_check=True)
                    nc.tensor.matmul(out=pB[:, 256:512], lhsT=wts1[64:124, 128:256],
                                     rhs=xB, start=True, stop=True,
                                     skip_group_check=True)
                    if w % 2 == 0:
                        pG = pg.tile([128, 256], F32, tag="g")
                        gpair = [pG, None]
                    else:
                        pG = gpair[0]
                    gbase = 64 * (w % 2)
                    nc.tensor.matmul(out=pG[gbase:gbase + 64, :],
                                     lhsT=wg[0:124, 0:64],
                                     rhs=xt[0:124, lc:lc + WC],
                                     start=True, stop=True,
                                     skip_group_check=True,
                                     tile_position=(0, gbase))
                    h1A = hs.tile([128, 512], h1dt, tag="h1A")
                    evict(nc, h1A, pA, None, 512)
                    h1B = hs.tile([128, 512], h1dt, tag="h1B")
                    evict(nc, h1B, pB, None, 512)
                    if w % 2 == 1:
                        gsb = hs.tile([128, 256], BF16, tag="G")
                        evict(nc, gsb, pG, None, 256)
                        gpair[1] = gsb
                        state[w - 1] = state[w - 1][:2] + (gpair,) + state[w - 1][3:]
                        state[w] = (h1A, h1B, gpair, None, None)
                    else:
                        state[w] = (h1A, h1B, gpair, None, None)

                # ---- preds/logits (flipped matmuls) for window w-4
                if 0 <= w - 4 < NW:
                    w2i = w - 4
                    _, _, gsb, h2a, h2b = state[w2i]
                    d = w2i // DRW
                    if w2i % DRW == 0:
                        sm_t = psm.tile([128, 256], F32, tag="sm")
                        smt[d] = sm_t
                    sm = smt[d]
                    gsb_t = gsb[1]
                    for sl in range(4):
                        g0 = ((w2i % DRW) * 4 + sl) * 8
                        cl = 128 * sl
                        nc.tensor.matmul(out=sm[:, g0:g0 + 2],
                                         lhsT=h2a[:, cl:cl + 128],
                                         rhs=sp[:, 0:2], start=True, stop=True,
                                         skip_group_check=True,
                                         tile_position=(0, 0))
                        nc.tensor.matmul(out=sm[:, g0 + 2:g0 + 4],
                                         lhsT=h2b[:, cl:cl + 128],
                                         rhs=sp[:, 2:4], start=True, stop=True,
                                         skip_group_check=True,
                                         tile_position=(0, 0))
                        gp = 64 * (w2i % 2) + (0 if sl < 2 else 32)
                        gc = 128 * (sl % 2)
                        nc.tensor.matmul(out=sm[:, g0 + 4:g0 + 8],
                                         lhsT=gsb_t[gp:gp + 32, gc:gc + 128],
                                         rhs=gw2[0:32, 4 * (sl // 2):4 * (sl // 2) + 4],
                                         start=True, stop=True,
                                         skip_group_check=True,
                                         tile_position=(0, 0))
                    del state[w2i]

                    # ---- combine drain once per DRW windows
                    if w2i % DRW == DRW - 1:
                        S = ds.tile([128, 256], F32, tag="S")
                        nc.gpsimd.tensor_tensor(S, sm, pat, op=ALU.add)
                        S4 = S.rearrange("p (g two f) -> p g two f", two=2, f=4)
                        EX = ds.tile([128, 128], F32, tag="EX")
                        nc.scalar.activation(EX, S4[:, :, 1, :], AF.Exp)
                        PW = ds.tile([128, 128], F32, tag="PW")
                        nc.gpsimd.tensor_tensor(PW, S4[:, :, 0, :], EX, op=ALU.mult)
                        NUM = ds.tile([128, 32], F32, tag="NUM")
                        nc.vector.tensor_reduce(
                            NUM, PW.rearrange("p (g f) -> p g f", f=4),
                            mybir.AxisListType.X, ALU.add)
                        DEN = ds.tile([128, 32], F32, tag="DEN")
                        nc.vector.tensor_reduce(
                            DEN, EX.rearrange("p (g f) -> p g f", f=4),
                            mybir.AxisListType.X, ALU.add)
                        REC = ds.tile([128, 32], F32, tag="REC")
                        nc.vector.reciprocal(REC, DEN)
                        OUT = ds.tile([128, 32], F32, tag="OUT")
                        nc.vector.tensor_tensor(OUT, NUM, REC, op=ALU.mult)
                        nc.sync.dma_start(out=out_d[d], in_=OUT)
                        del smt[d]

    if not nc.is_finalized():
        nc.finalize()
    return nc


def _pack_host(variant, w1, b1, bn1_g, bn1_b, bn1_m, bn1_v, w2, b2, bn2_g,
               bn2_b, bn2_m, bn2_v, w3, b3, wp, bp, gw1, gb1, gw2, gb2):
    f = np.float32
    s1 = (bn1_g / np.sqrt(bn1_v + EPS)).astype(f)               # (E,H)
    W1e = (w1 * s1[:, None, :]).astype(f)                       # (E,IN,H)
    c1 = ((b1 - bn1_m) * s1 + bn1_b).astype(f)                  # (E,H)
    s2f = (bn2_g / np.sqrt(bn2_v + EPS)).astype(f)
    W2e = (w2 * s2f[:, None, :]).astype(f)                      # (E,H,H)
    c2 = ((b2 - bn2_m) * s2f + bn2_b).astype(f)                 # (E,H)
    wep = np.einsum("ehm,em->eh", w3, wp).astype(f)             # (E,H)
    bep = (np.einsum("em,em->e", b3, wp) + bp).astype(f)        # (E,)
    aw = np.abs(wep)
    sg = np.sign(wep).astype(f)
    W2p = (W2e * aw[:, None, :]).astype(f)                      # (E,H,H)
    c2p = (c2 * aw).astype(f)                                   # (E,H)

    wts1 = np.zeros((128, 256), f)
    for blk, (ea, eb) in enumerate(((0, 1), (2, 3))):
        c0 = 128 * blk
        for base in (0, 64):
            wts1[base:base + 59, c0:c0 + 64] = W1e[ea]
            wts1[base:base + 59, c0 + 64:c0 + 128] = W1e[eb]
            wts1[base + 59, c0:c0 + 64] = c1[ea]
            wts1[base + 59, c0 + 64:c0 + 128] = c1[eb]

    wgp = np.zeros((128, 64), f)
    wgp[0:59, 0:32] = gw1
    wgp[59, 0:32] = gb1
    wgp[64:123, 32:64] = gw1
    wgp[123, 32:64] = gb1

    if variant == "bf16":
        po = [0, 1, 2, 3]
        w2b = np.zeros((128, 256), f)
        w2b[0:64, 0:64] = W2p[0]
        w2b[64:128, 64:128] = W2p[1]
        w2b[0:64, 128:192] = W2p[2]
        w2b[64:128, 192:256] = W2p[3]
        c2sv = np.stack([np.concatenate([c2p[0], c2p[1]]),
                         np.concatenate([c2p[2], c2p[3]])], axis=1)
        spv = np.zeros((128, 4), f)
        spv[0:64, 0] = sg[0]
        spv[64:128, 1] = sg[1]
        spv[0:64, 2] = sg[2]
        spv[64:128, 3] = sg[3]
    else:
        po = [0, 2, 1, 3]
        # keep fp8 weights out of the e4m3 subnormal zone; the 2**-10 is
        # folded into the +/-1 pred patterns (exact in bf16)
        FSC = 1024.0
        W2p = W2p * FSC
        c2p = c2p * FSC
        sg = sg / FSC
        t02 = np.zeros((128, 2, 128), f)
        t02[0:64, 0, 0:64] = W2p[0]
        t02[0:64, 1, 64:128] = W2p[2]
        t13 = np.zeros((128, 2, 128), f)
        t13[64:128, 0, 0:64] = W2p[1]
        t13[64:128, 1, 64:128] = W2p[3]
        w2b = np.concatenate([t02.reshape(128, 256), t13.reshape(128, 256)],
                             axis=1)
        c2sv = np.stack([np.concatenate([c2p[0], c2p[2]]),
                         np.concatenate([c2p[1], c2p[3]])], axis=1)
        spv = np.zeros((128, 4), f)
        spv[0:64, 0] = sg[0]
        spv[64:128, 1] = sg[2]
        spv[0:64, 2] = sg[1]
        spv[64:128, 3] = sg[3]

    gw2t = np.zeros((32, 8), f)
    gw2t[:, 0:4] = gw2[:, po]
    gw2t[:, 4:8] = gw2[:, po]

    grp = np.concatenate([bep[po], gb2[po]]).astype(f)          # (8,)
    patv = np.tile(grp, 32)[None, :].repeat(128, axis=0)

    w2dt = ml_dtypes.bfloat16 if variant == "bf16" else ml_dtypes.float8_e4m3fn
    return dict(
        wts1=np.ascontiguousarray(wts1),
        wg=np.ascontiguousarray(wgp),
        w2b=np.ascontiguousarray(w2b.astype(w2dt)),
        c2s=np.ascontiguousarray(c2sv.astype(f)),
        sp=np.ascontiguousarray(spv.astype(ml_dtypes.bfloat16)),
        gw2t=np.ascontiguousarray(gw2t.astype(ml_dtypes.bfloat16)),
        pat=np.ascontiguousarray(patv),
    )


def _pack_x_core(xc):
    # xc: (BC, 59) float32 -> feature-major image (128, HB)
    xi = np.zeros((128, HB), np.float32)
    xi[0:59] = xc[:HB].T
    xi[59] = 1.0
    xi[64:123] = xc[HB:].T
    xi[123] = 1.0
    return np.ascontiguousarray(xi)


def _unshard_core(o):
    # o: (ND, 128, 32) -> (BC,) predictions
    v = o.reshape(ND, 128, DRW, 4).transpose(0, 2, 3, 1)  # (d, wl, sl, p)
    a = v[:, :, 0:2, :].reshape(HB)
    b = v[:, :, 2:4, :].reshape(HB)
    return np.concatenate([a, b])


def kernel(**inputs):
    x = np.asarray(inputs["x"], dtype=np.float32)
    wk = {k: np.asarray(v, dtype=np.float32) for k, v in inputs.items()
          if k != "x"}
    packed = _pack_host(VARIANT, **wk)

    key = "nc_" + VARIANT
    if key not in _CACHE:
        _CACHE[key] = _build(VARIANT)
    nc = _CACHE[key]

    in_maps = []
    for c in range(NCORES):
        m = {"x": _pack_x_core(x[c * BC:(c + 1) * BC])}
        m.update(packed)
        in_maps.append(m)
    _CACHE["in_maps"] = in_maps

    res = run_bass_kernel_spmd(nc, in_maps, core_ids=list(range(NCORES)))
    _CACHE["last"] = res
    outs = [_unshard_core(r["out"]) for r in res.results]
    return np.concatenate(outs).reshape(B, 1).astype(np.float32)
